# revision 44
# baseline (speedup 1.0000x reference)
"""DSTMamba Trainium2 kernel: 8 NeuronCores, SPMD. v2.

Core c handles (batch b=c//2, direction d=c%2); odd cores see the token
axis reversed so one forward-scan program serves both directions. The
bidirectional merge is a pair AllGather (bf16) + reversal-linearity
combine (symmetric SPMD, no control flow).

v2 vs v1:
- all data DMAs issued from the SP engine (HWDGE) instead of gpsimd
  (SWDGE costs ~1us of Pool time per DMA); weights host-blocked to
  [128, X] so each tensor is ONE DMA.
- depthwise causal conv folded into in_proj: host premultiplies
  diag(cw1)@W / diag(cw0)@W; the t-1 tap is a second PSUM-accumulated
  matmul on a token-shifted rhs slice. Act Silu evacuates PSUM directly.
- B/C state rows broadcast in 2 wide DMAs per half-layer ([128, 8N])
  instead of 32 per-state broadcasts.
- y = D*xc + sum_s h_s*C_s accumulated on the idle TensorEngine
  (diag(D) matmul + per-state identity matmuls into PSUM).
- pair exchange: bf16 AllGather + merge xnew = x + fT + rev(G0+G1-fT);
  tail trend extraction emitted inside the exchange gaps.
- activation stream in bf16 (weights too) for 2x DVE ops and lighter
  DMA/SBUF; dA stays f32 (bf16 would corrupt near-1 decay factors).
"""

import numpy as np

import concourse.bacc as bacc
import concourse.mybir as mybir
from concourse import tile
from concourse.bass_utils import run_bass_kernel_spmd

B, L, H, N = 4, 512, 96, 862
DM, DS = 256, 16
DI = 512
DTR = 16
DFF, NLAYERS = 256, 2
DSL, KSTD = 3, 25
EPS = 1e-5

F32 = mybir.dt.float32
F32R = mybir.dt.float32r
BF16 = mybir.dt.bfloat16
AL = mybir.AluOpType
AF = mybir.ActivationFunctionType

NC2 = [(0, 512), (512, 350)]  # psum-bank-sized moving-dim chunks of N=862
PAIRS = [[0, 1], [2, 3], [4, 5], [6, 7]]

DEBUG = False
TRUNC = False  # build only front + layer-0 in_proj (debug bisection)
_CACHE = {}


# ---------------------------------------------------------------- host math
def _mavg_matrix(length):
    M = np.zeros((length, length), np.float64)
    p = (KSTD - 1) // 2
    for i in range(length):
        for d in range(-p, p + 1):
            j = min(max(i + d, 0), length - 1)
            M[i, j] += 1.0 / KSTD
    return M


def _pool_matrix(lo, hi):
    P = np.zeros((lo, hi), np.float64)
    for i in range(lo):
        P[i, 2 * i] = 0.5
        P[i, 2 * i + 1] = 0.5
    return P


def _trend_ops():
    ops = []
    P = np.eye(L)
    cur = L
    for s in range(DSL + 1):
        ops.append(_mavg_matrix(cur) @ P)
        if s < DSL:
            P = _pool_matrix(cur // 2, cur) @ P
            cur //= 2
    return ops  # [512,512],[256,512],[128,512],[64,512]


def _col(v):
    v = np.asarray(v, np.float32).reshape(-1)
    if v.size <= 128:
        return np.ascontiguousarray(v.reshape(-1, 1))
    return np.ascontiguousarray(v.reshape(-1, 128).T)


def _row(v):
    return np.ascontiguousarray(np.asarray(v, np.float32).reshape(1, -1))


def _t(m):
    return np.ascontiguousarray(np.asarray(m, np.float32).T)


def _bf(x):
    import ml_dtypes
    return np.ascontiguousarray(np.asarray(x, dtype=ml_dtypes.bfloat16))


def _blk(wT, bf=False):
    """[K, M] lhsT -> [min(K,128), ceil(K/128)*M] column-blocked SBUF image."""
    wT = np.asarray(wT, np.float32)
    K, M = wT.shape
    if K > 128:
        assert K % 128 == 0
        wT = np.concatenate([wT[b * 128:(b + 1) * 128] for b in range(K // 128)],
                            axis=1)
    wT = np.ascontiguousarray(wT)
    return _bf(wT) if bf else wT


def make_core_inputs(inputs, core):
    b, d = core // 2, core % 2
    g = lambda k: np.asarray(inputs[k], np.float32)

    m = {}
    x = g("history_data")[b, :, :, 0]
    if d == 1:
        x = x[:, ::-1]
    m["x_in"] = _blk(np.ascontiguousarray(x))  # [128, 4N]

    tops = _trend_ops()
    m["seaop_T"] = _blk(_t(np.eye(L) - tops[0]))
    for s in range(4):
        m[f"trop{s}_T"] = _blk(_t(tops[s]))

    m["emb_lhsT"] = _blk(_t(g("emb_w")))
    m["emb_b"] = _col(g("emb_b"))

    for l in range(NLAYERS):
        W = g("m_in")[l, d]               # [1024, 256]; rows 0:512 xc, 512: z
        cw = g("m_conv_w")[l, d]          # [512, 2]
        Wxc, Wz = W[:DI], W[DI:]
        W1 = Wxc * cw[:, 1:2]             # diag(cw1) @ Wxc
        W0 = Wxc * cw[:, 0:1]             # diag(cw0) @ Wxc
        m[f"inT_{l}"] = _blk(_t(np.concatenate([W1, Wz], 0)), bf=True)
        m[f"in0T_{l}"] = _blk(_t(W0), bf=True)
        m[f"cb_{l}"] = _col(g("m_conv_b")[l, d])

        xp = g("m_xproj")[l, d]           # [48, 512]: dt 0:16, B 16:32, C 32:48
        xp_r = np.concatenate([xp[16:32], xp[32:48], xp[0:16]], 0)  # B|C|dt
        m[f"xpT_{l}"] = _blk(_t(xp_r), bf=True)                  # [128, 4*48]
        m[f"dtwT_{l}"] = _blk(_t(g("m_dt_w")[l, d]), bf=True)    # [16, 512]
        m[f"dtb_{l}"] = _col(g("m_dt_b")[l, d])

        Dv = g("m_D")[l, d]
        dg = np.zeros((128, 4 * 128), np.float32)
        for gi in range(4):
            dg[:, gi * 128:(gi + 1) * 128] = np.diag(Dv[gi * 128:(gi + 1) * 128])
        m[f"diagD_{l}"] = _bf(dg)

        m[f"outT_{l}"] = _blk(_t(g("m_out")[l, d]), bf=True)     # [128, 4*256]
        m[f"n1w_{l}"] = _col(g("n1_w")[l]); m[f"n1b_{l}"] = _col(g("n1_b")[l])
        m[f"n2w_{l}"] = _col(g("n2_w")[l]); m[f"n2b_{l}"] = _col(g("n2_b")[l])
        m[f"f1T_{l}"] = _blk(_t(g("f1_w")[l]), bf=True)          # [128, 2*256]
        m[f"f1b_{l}"] = _col(g("f1_b")[l])
        m[f"f2T_{l}"] = _blk(_t(g("f2_w")[l]), bf=True)
        m[f"f2b_{l}"] = _col(g("f2_b")[l])

    m["I128"] = _bf(np.eye(128))
    m["encnw"] = _col(g("encn_w")); m["encnb"] = _col(g("encn_b"))
    m["proj_lhsT"] = _blk(_t(g("proj_w")), bf=True)              # [128, 2*96]
    m["projb"] = _col(g("proj_b"))

    for i in range(DSL):
        m[f"u{i}w1T"] = _blk(_t(g(f"u{i}w1")), bf=True)
        m[f"u{i}b1"] = _col(g(f"u{i}b1"))
        m[f"u{i}w2T"] = _blk(_t(g(f"u{i}w2")), bf=True)
        m[f"u{i}b2"] = _col(g(f"u{i}b2"))
    for s in range(4):
        m[f"map{s}T"] = _blk(_t(g(f"map{s}_w")), bf=True)
    m["mapb"] = _col(sum(g(f"map{s}_b") for s in range(4)))

    rvw, rvb, trw = g("revin_w"), g("revin_b"), g("tre_w")
    if d == 1:
        rvw, rvb, trw = rvw[::-1], rvb[::-1], trw[::-1]
    m["rvw_row"] = _row(rvw)
    m["rvb_row"] = _row(rvb)
    m["trw_row"] = _row(trw)
    m["ones_col"] = np.ones((128, 1), np.float32)
    m["ones_bf"] = _bf(np.ones((128, 1)))
    return m


# ------------------------------------------------------------- device build
class _Ctx:
    pass


def _build():
    nc = bacc.Bacc("TRN2", target_bir_lowering=False, debug=False,
                   num_devices=8)

    def din(name, shape, dt=F32):
        return nc.dram_tensor(name, list(shape), dt, kind="ExternalInput").ap()

    I = {}
    I["x_in"] = din("x_in", [128, 4 * N], F32R)
    I["seaop_T"] = din("seaop_T", [128, 4 * L], F32R)
    for s, ls in enumerate([512, 256, 128, 64]):
        I[f"trop{s}_T"] = din(f"trop{s}_T", [128, 4 * ls], F32R)
    I["emb_lhsT"] = din("emb_lhsT", [128, 4 * DM], F32R)
    I["emb_b"] = din("emb_b", [128, DM // 128])
    for l in range(NLAYERS):
        I[f"inT_{l}"] = din(f"inT_{l}", [128, 2 * 1024], BF16)
        I[f"in0T_{l}"] = din(f"in0T_{l}", [128, 2 * 512], BF16)
        I[f"cb_{l}"] = din(f"cb_{l}", [128, DI // 128])
        I[f"xpT_{l}"] = din(f"xpT_{l}", [128, 4 * 48], BF16)
        I[f"dtwT_{l}"] = din(f"dtwT_{l}", [16, 512], BF16)
        I[f"dtb_{l}"] = din(f"dtb_{l}", [128, DI // 128])
        I[f"diagD_{l}"] = din(f"diagD_{l}", [128, 4 * 128], BF16)
        I[f"outT_{l}"] = din(f"outT_{l}", [128, 4 * DM], BF16)
        for k in ["n1w", "n1b", "n2w", "n2b", "f1b", "f2b"]:
            I[f"{k}_{l}"] = din(f"{k}_{l}", [128, DM // 128])
        I[f"f1T_{l}"] = din(f"f1T_{l}", [128, 2 * DFF], BF16)
        I[f"f2T_{l}"] = din(f"f2T_{l}", [128, 2 * DM], BF16)
    I["I128"] = din("I128", [128, 128], BF16)
    I["encnw"] = din("encnw", [128, DM // 128])
    I["encnb"] = din("encnb", [128, DM // 128])
    I["proj_lhsT"] = din("proj_lhsT", [128, 2 * H], BF16)
    I["projb"] = din("projb", [H, 1])
    for i, (li, lo) in enumerate([(64, 128), (128, 256), (256, 512)]):
        I[f"u{i}w1T"] = din(f"u{i}w1T", [min(li, 128), max(1, li // 128) * lo],
                            BF16)
        I[f"u{i}b1"] = din(f"u{i}b1", [min(lo, 128), max(1, lo // 128)])
        I[f"u{i}w2T"] = din(f"u{i}w2T", [min(lo, 128), max(1, lo // 128) * lo],
                            BF16)
        I[f"u{i}b2"] = din(f"u{i}b2", [min(lo, 128), max(1, lo // 128)])
    for s, ls in enumerate([512, 256, 128, 64]):
        I[f"map{s}T"] = din(f"map{s}T", [min(ls, 128), max(1, ls // 128) * H],
                            BF16)
    I["mapb"] = din("mapb", [H, 1])
    for k in ["rvw_row", "rvb_row", "trw_row"]:
        I[k] = din(k, [1, N])
    I["ones_col"] = din("ones_col", [128, 1], F32R)
    I["ones_bf"] = din("ones_bf", [128, 1], BF16)

    out_pred = nc.dram_tensor("pred", [H, N], F32, kind="ExternalOutput").ap()

    c = _Ctx()
    c.nc, c.I, c.out_pred = nc, I, out_pred
    c.dbg = {}
    with tile.TileContext(nc) as tc:
        c.tc = tc
        _emit(c)
    nc.compile()
    return nc


def _dbg(c, name, aps):
    if not DEBUG:
        return
    nc = c.nc
    rows = sum(a.shape[0] for a in aps)
    o = nc.dram_tensor(f"dbg_{name}", [rows, N], F32, kind="ExternalOutput").ap()
    r0 = 0
    for a in aps:
        r = a.shape[0]
        nc.gpsimd.dma_start(o[r0:r0 + r, :], a)
        r0 += r
    c.dbg[name] = o


def _load(c, pool, key, tag=None):
    ap = c.I[key]
    t_ = pool.tile(list(ap.shape), ap.dtype, name=key, tag=tag or key)
    c.nc.sync.dma_start(t_[:, :], ap[:, :])
    return t_


def _lhs(tile_, kb, mo, M, mw=128):
    """k-block kb, m-cols [mo, mo+mw) slice of a column-blocked lhsT tile.
    ONLY for f32r weights — bf16 lhsT column slices at nonzero offsets
    fault the PE; bf16 weights go through _load_mt instead."""
    return tile_[:, kb * M + mo: kb * M + mo + mw]


def _matsum_w(c, psum, wtile, M, nk, mo, mw, rhs, n0, nl, start=True, stop=True):
    """psum (+)= sum_kb lhsT(kb)[mo:mo+mw].T @ rhs[kb][:, n0:n0+nl]"""
    nc = c.nc
    for kb in range(nk):
        nc.tensor.matmul(psum[:, :], _lhs(wtile, kb, mo, M, mw),
                         rhs[kb][:, n0:n0 + nl],
                         start=(start and kb == 0), stop=(stop and kb == nk - 1))


def _load_mt(c, pool, key, M, nk, tag=None):
    """Column-blocked dram [K<=128, nk*M] -> tiles[kb][mc] of [K, <=128],
    one DMA each, so every bf16 lhsT operand sits at column offset 0."""
    ap = c.I[key]
    K = ap.shape[0]
    out = []
    for kb in range(nk):
        row = []
        for mc in range((M + 127) // 128):
            mw = min(128, M - mc * 128)
            o0 = kb * M + mc * 128
            t_ = pool.tile([K, mw], ap.dtype, name=f"{key}_{kb}_{mc}",
                           tag=f"{tag or key}_{kb}_{mc}")
            c.nc.sync.dma_start(t_[:, :], ap[:, o0:o0 + mw])
            row.append(t_)
        out.append(row)
    return out


def _matsum_t(c, psum, wt, mc, rhs, n0, nl, start=True, stop=True):
    """psum (+)= sum_kb wt[kb][mc].T @ rhs[kb][:, n0:n0+nl]"""
    nc = c.nc
    nk = len(wt)
    for kb in range(nk):
        nc.tensor.matmul(psum[:, :], wt[kb][mc][:, :],
                         rhs[kb][:, n0:n0 + nl],
                         start=(start and kb == 0), stop=(stop and kb == nk - 1))


def _bcast(c, pool, row_ap, parts, tag, via_dram=True, bufs=1, dt=F32):
    # broadcast_to (stride-0 partition) DMAs must go through SWDGE
    # (gpsimd): the HWDGE path corrupts SBUF with late/incomplete writes.
    nc = c.nc
    if via_dram:
        d = c.dp.tile([1, N], F32, name=f"bd_{tag}", tag=f"bd_{tag}")
        nc.sync.dma_start(d[:, :], row_ap.bitcast(F32))
        src = d[:, :]
    else:
        src = row_ap.bitcast(F32)
    bt = pool.tile([parts, N], dt, name=f"bc_{tag}", tag=f"bc_{tag}",
                   bufs=bufs)
    nc.gpsimd.dma_start(bt[:, :], src.broadcast_to([parts, N]))
    return bt


def _layer_norm(c, scr, xin, wcol, bcol, outpool, outtag):
    """xin: 2 [128,N] bf16 tiles -> 2 [128,N] bf16 tiles (norm over 256)."""
    nc, pm = c.nc, c.pm
    mrow = scr.tile([1, N], F32, name=f"lnm_{outtag}", tag="ln_mrow", bufs=1)
    qrow = scr.tile([1, N], F32, name=f"lnq_{outtag}", tag="ln_qrow", bufs=1)
    for n0, nl in NC2:
        ps = pm.tile([1, nl], F32, name="lnps", tag="mm1")
        for mi in range(2):
            nc.tensor.matmul(ps[:, :], c.ones_bf[:, :], xin[mi][:, n0:n0 + nl],
                             start=(mi == 0), stop=(mi == 1))
        nc.scalar.activation(mrow[:, n0:n0 + nl], ps[:, :], AF.Copy,
                             scale=1.0 / DM)
        ps2 = pm.tile([1, nl], F32, name="lnps2", tag="mm1")
        for mi in range(2):
            sq = scr.tile([128, N], BF16, name="lnsq", tag="sq", bufs=2)
            nc.scalar.activation(sq[:, n0:n0 + nl],
                                 xin[mi][:, n0:n0 + nl], AF.Square)
            nc.tensor.matmul(ps2[:, :], c.ones_bf[:, :], sq[:, n0:n0 + nl],
                             start=(mi == 0), stop=(mi == 1))
        nc.scalar.activation(qrow[:, n0:n0 + nl], ps2[:, :], AF.Copy,
                             scale=1.0 / DM)
    tmp_ = scr.tile([1, N], F32, name=f"lnt_{outtag}", tag="ln_trow", bufs=1)
    nc.vector.tensor_mul(tmp_[:, :], mrow[:, :], mrow[:, :])
    nc.vector.tensor_sub(qrow[:, :], qrow[:, :], tmp_[:, :])
    nc.scalar.activation(qrow[:, :], qrow[:, :], AF.Ln, bias=c.epscol[:1, :])
    nc.scalar.activation(qrow[:, :], qrow[:, :], AF.Exp, scale=-0.5)
    mb = _bcast(c, scr, mrow[:, :], 128, "lnm", dt=BF16)
    rb = _bcast(c, scr, qrow[:, :], 128, "lnr", dt=BF16)
    out = []
    for mi in range(2):
        o = outpool.tile([128, N], BF16, name=f"{outtag}{mi}", tag=f"{outtag}{mi}")
        d1 = scr.tile([128, N], BF16, name="lnd1", tag="d1", bufs=2)
        nc.vector.tensor_sub(d1[:, :], xin[mi][:, :], mb[:, :])
        nc.vector.tensor_mul(d1[:, :], d1[:, :], rb[:, :])
        nc.vector.tensor_scalar(o[:, :], d1[:, :],
                                wcol[:, mi:mi + 1],
                                bcol[:, mi:mi + 1], AL.mult, AL.add)
        out.append(o)
    return out


def _emit(c):
    nc, tc, I = c.nc, c.tc, c.I
    import contextlib
    with contextlib.ExitStack() as est:
        gp = est.enter_context(tc.tile_pool(name="glob", bufs=1))
        pm = est.enter_context(tc.tile_pool(name="pmm", bufs=2, space="PSUM"))
        dp = est.enter_context(tc.tile_pool(name="drm", bufs=1, space="DRAM"))
        c.gp, c.pm, c.dp = gp, pm, dp

        c.ones_col = _load(c, gp, "ones_col")
        c.ones_bf = _load(c, gp, "ones_bf")
        c.I128 = _load(c, gp, "I128")
        epscol = gp.tile([128, 1], F32, name="epscol", tag="epscol")
        c.nc.gpsimd.memset(epscol[:, :], EPS)
        c.epscol = epscol
        r_mean = gp.tile([1, N], F32, name="r_mean", tag="r_mean")
        r_sc = gp.tile([1, N], F32, name="r_sc", tag="r_sc")
        c.r_mean, c.r_sc = r_mean, r_sc

        # ======================================================== front
        with tc.tile_pool(name="front", bufs=1) as fp:
            r_std = fp.tile([1, N], F32, name="r_std", tag="r_std")
            r_wr = fp.tile([1, N], F32, name="r_wr", tag="r_wr")
            r_msq = fp.tile([1, N], F32, name="r_msq", tag="r_msq")
            Xw = _load(c, fp, "x_in")

            def Xs(ci, a, b):
                return Xw[:, ci * N + a: ci * N + b]

            for n0, nl in NC2:
                ps = pm.tile([1, nl], F32, name="rvs", tag="mm1")
                for ci in range(4):
                    nc.tensor.matmul(ps[:, :], c.ones_col[:, :],
                                     Xs(ci, n0, n0 + nl),
                                     start=(ci == 0), stop=(ci == 3))
                nc.scalar.activation(r_mean[:, n0:n0 + nl], ps[:, :],
                                     AF.Copy, scale=1.0 / L)
                ps2 = pm.tile([1, nl], F32, name="rvq", tag="mm1")
                for ci in range(4):
                    sq = fp.tile([128, N], F32R, name="rvsq", tag="fsq", bufs=2)
                    nc.scalar.activation(sq[:, n0:n0 + nl],
                                         Xs(ci, n0, n0 + nl).bitcast(F32),
                                         AF.Square)
                    nc.tensor.matmul(ps2[:, :], c.ones_col[:, :],
                                     sq[:, n0:n0 + nl],
                                     start=(ci == 0), stop=(ci == 3))
                nc.scalar.activation(r_msq[:, n0:n0 + nl], ps2[:, :],
                                     AF.Copy, scale=1.0 / L)
            nc.vector.tensor_mul(r_wr[:, :], r_mean[:, :], r_mean[:, :])
            nc.vector.tensor_sub(r_msq[:, :], r_msq[:, :], r_wr[:, :])
            nc.scalar.activation(r_msq[:, :], r_msq[:, :], AF.Ln,
                                 bias=c.epscol[:1, :])
            nc.scalar.activation(r_std[:, :], r_msq[:, :], AF.Exp, scale=0.5)
            nc.scalar.activation(r_wr[:, :], r_msq[:, :], AF.Exp, scale=-0.5)
            rvw = fp.tile([1, N], F32, name="rvwrow", tag="rvwrow")
            nc.sync.dma_start(rvw[:, :], I["rvw_row"][:, :])
            nc.vector.tensor_mul(r_wr[:, :], r_wr[:, :], rvw[:, :])
            t1 = fp.tile([1, N], F32, name="sct1", tag="sct1")
            nc.vector.tensor_scalar_add(t1[:, :], rvw[:, :], 1e-10)
            nc.vector.reciprocal(t1[:, :], t1[:, :])
            nc.vector.tensor_mul(r_sc[:, :], t1[:, :], r_std[:, :])

            mb = _bcast(c, fp, r_mean[:, :], 128, "rvm")
            wb = _bcast(c, fp, r_wr[:, :], 128, "rvw")
            bb = _bcast(c, fp, I["rvb_row"], 128, "rvb", via_dram=False)
            c.xn = []
            for ci in range(4):
                o = gp.tile([128, N], F32R, name=f"xn{ci}", tag=f"xn{ci}")
                d1 = fp.tile([128, N], F32, name="rvd", tag="rvd", bufs=2)
                nc.vector.tensor_sub(d1[:, :], Xs(ci, 0, N).bitcast(F32),
                                     mb[:, :])
                nc.vector.tensor_mul(d1[:, :], d1[:, :], wb[:, :])
                nc.vector.tensor_add(o[:, :], d1[:, :], bb[:, :])
                c.xn.append(o)
            _dbg(c, "xn", [t[:, :].bitcast(F32) for t in c.xn])

            SE = _load(c, fp, "seaop_T")
            xsea = []
            for mc in range(4):
                t_ = fp.tile([128, N], F32R, name=f"xsea{mc}", tag=f"xsea{mc}")
                xsea.append(t_)
                for n0, nl in NC2:
                    ps = pm.tile([128, nl], F32, name="semm", tag="mm")
                    _matsum_w(c, ps, SE, L, 4, mc * 128, 128, c.xn, n0, nl)
                    nc.scalar.copy(t_[:, n0:n0 + nl], ps[:, :])
            EL = _load(c, fp, "emb_lhsT")
            emb_b = _load(c, fp, "emb_b")
            xt = []
            for mc in range(2):
                t_ = gp.tile([128, N], BF16, name=f"xtA{mc}", tag=f"xtA{mc}")
                xt.append(t_)
                for n0, nl in NC2:
                    ps = pm.tile([128, nl], F32, name="embmm", tag="mm")
                    _matsum_w(c, ps, EL, DM, 4, mc * 128, 128, xsea, n0, nl)
                    nc.scalar.activation(t_[:, n0:n0 + nl], ps[:, :],
                                         AF.Identity,
                                         bias=emb_b[:, mc:mc + 1])
            _dbg(c, "x0", [t[:, :] for t in xt])

        # ======================================================== encoder
        c.trt = None
        for l in range(1 if TRUNC else NLAYERS):
            with contextlib.ExitStack() as lst:
                lp = lst.enter_context(tc.tile_pool(name=f"lay{l}", bufs=1))
                rp = lst.enter_context(tc.tile_pool(name=f"rot{l}", bufs=2))
                pa = lst.enter_context(
                    tc.tile_pool(name=f"pda{l}", bufs=2, space="PSUM"))
                xt = _mamba_layer(c, l, lp, rp, pa, xt)
                if l == 0:
                    _dbg(c, "xl0", [t[:, :] for t in xt])

        # ======================================================== tail
        if TRUNC:
            return
        with contextlib.ExitStack() as tst:
            tp = tst.enter_context(tc.tile_pool(name="tail", bufs=1))
            encw = _load(c, tp, "encnw")
            encb = _load(c, tp, "encnb")
            xf = _layer_norm(c, tp, xt, encw, encb, c.gp, "xtB")
            PRJ = _load_mt(c, tp, "proj_lhsT", H, 2)
            projb = _load(c, tp, "projb")
            seaT = tp.tile([H, N], F32, name="seaT", tag="seaT")
            for n0, nl in NC2:
                ps = pm.tile([H, nl], F32, name="prmm", tag="mm")
                _matsum_t(c, ps, PRJ, 0, xf, n0, nl)
                nc.scalar.activation(seaT[:, n0:n0 + nl], ps[:, :], AF.Identity,
                                     bias=projb[:, :])
            _dbg(c, "sea", [seaT[:, :]])

            tr0, tr1, tr2, tr3 = c.trt
            o1, o2 = c.mix_o1, c.mix_o2
            o3 = _mixstep(c, tp, o2, 2, tr0)

            outst = [o3, o2, o1, tr3]
            MP = [_load_mt(c, tp, f"map{s}T", H, len(outst[s]))
                  for s in range(4)]
            mapb = _load(c, tp, "mapb")
            treT = tp.tile([H, N], F32, name="treT", tag="treT")
            for n0, nl in NC2:
                ps = pm.tile([H, nl], F32, name="mpmm", tag="mm")
                ops = []
                for s in range(4):
                    for kb in range(len(outst[s])):
                        ops.append((MP[s][kb][0], outst[s][kb]))
                for i, (w_, x_) in enumerate(ops):
                    nc.tensor.matmul(ps[:, :], w_[:, :], x_[:, n0:n0 + nl],
                                     start=(i == 0), stop=(i == len(ops) - 1))
                nc.scalar.activation(treT[:, n0:n0 + nl], ps[:, :], AF.Identity,
                                     bias=mapb[:, :])
            _dbg(c, "tre", [treT[:, :]])

            p1 = tp.tile([H, N], F32, name="fin1", tag="fin1")
            twb = _bcast(c, tp, I["trw_row"], H, "finb", via_dram=False)
            nc.vector.tensor_mul(p1[:, :], treT[:, :], twb[:, :])
            nc.vector.tensor_add(p1[:, :], p1[:, :], seaT[:, :])
            rbb = _bcast(c, tp, I["rvb_row"], H, "finb", via_dram=False)
            nc.vector.tensor_sub(p1[:, :], p1[:, :], rbb[:, :])
            scb = _bcast(c, tp, c.r_sc[:, :], H, "finb")
            nc.vector.tensor_mul(p1[:, :], p1[:, :], scb[:, :])
            mnb = _bcast(c, tp, c.r_mean[:, :], H, "finb")
            nc.vector.tensor_add(p1[:, :], p1[:, :], mnb[:, :])
            nc.sync.dma_start(c.out_pred[:, :], p1[:, :])


def _mixstep(c, gtpool, low, i, high):
    """TimeMixer trend mixing step i: high += W2 @ gelu(W1 @ low + b1) + b2."""
    nc, pm = c.nc, c.pm
    with c.tc.tile_pool(name=f"wu{i}", bufs=1) as wu:
        nk1 = len(low)
        lo_cols = c.I[f"u{i}w1T"].shape[1] // nk1
        W1 = _load_mt(c, wu, f"u{i}w1T", lo_cols, nk1)
        b1 = _load(c, wu, f"u{i}b1")
        W2 = _load_mt(c, wu, f"u{i}w2T", lo_cols, (lo_cols + 127) // 128)
        b2 = _load(c, wu, f"u{i}b2")
        gt = []
        for mc in range((lo_cols + 127) // 128):
            parts = min(128, lo_cols - mc * 128)
            g_ = gtpool.tile([parts, N], BF16, name=f"mxg{i}_{mc}",
                             tag=f"gA{mc}")
            gt.append(g_)
            for n0, nl in NC2:
                ps = pm.tile([parts, nl], F32, name="mxmm", tag="mm")
                _matsum_t(c, ps, W1, mc, low, n0, nl)
                nc.scalar.activation(g_[:, n0:n0 + nl], ps[:, :], AF.Gelu,
                                     bias=b1[:parts, mc:mc + 1])
        out = []
        for mc in range(len(high)):
            parts = high[mc].shape[0]
            o_ = high[mc]
            out.append(o_)
            for n0, nl in NC2:
                ps = pm.tile([parts, nl], F32, name="mxmm2", tag="mm")
                _matsum_t(c, ps, W2, mc, gt, n0, nl)
                b_ = gtpool.tile([parts, N], F32, name="mxb", tag="mxb",
                                 bufs=1)
                nc.scalar.activation(b_[:, n0:n0 + nl], ps[:, :], AF.Identity,
                                     bias=b2[:parts, mc:mc + 1])
                nc.vector.tensor_add(o_[:, n0:n0 + nl], o_[:, n0:n0 + nl],
                                     b_[:, n0:n0 + nl])
        return out


def _trend_extract(c, l):
    """Emit tail work that depends only on c.xn inside the exchange gaps.
    Layer 0's gap: trend scales 1..3. Layer 1's: scale 0 + mixsteps 0,1."""
    nc = c.nc
    if c.trt is None:
        c.trt = [None] * 4
    for s, ls in ([(1, 256), (2, 128), (3, 64)] if l == 0 else [(0, 512)]):
        with c.tc.tile_pool(name=f"wtr{s}", bufs=1) as wtr:
            TR = _load(c, wtr, f"trop{s}_T")
            mt = []
            for mc in range((ls + 127) // 128):
                parts = min(128, ls - mc * 128)
                t_ = c.gp.tile([parts, N], BF16, name=f"tr{s}_{mc}",
                               tag=f"tr{s}_{mc}")
                mt.append(t_)
                for n0, nl in NC2:
                    ps = c.pm.tile([parts, nl], F32, name="trmm", tag="mm")
                    _matsum_w(c, ps, TR, ls, 4, mc * 128, parts, c.xn, n0, nl)
                    nc.scalar.copy(t_[:, n0:n0 + nl], ps[:, :])
            c.trt[s] = mt
    if l == 1:
        c.mix_o1 = _mixstep(c, c.gp, c.trt[3], 0, c.trt[2])
        c.mix_o2 = _mixstep(c, c.gp, c.mix_o1, 1, c.trt[1])


def _mamba_layer(c, l, lp, rp, pa, xt):
    nc, pm = c.nc, c.pm

    # ---- in_proj with folded conv; Act Silu evacuates PSUM directly
    xcs, zr = [], []
    with c.tc.tile_pool(name=f"w1_{l}", bufs=1) as wp1:
        inT = _load_mt(c, wp1, f"inT_{l}", 1024, 2, tag="inT")
        in0T = _load_mt(c, wp1, f"in0T_{l}", 512, 2, tag="in0T")
        cb = _load(c, lp, f"cb_{l}")
        for f in range(8):
            is_xc = f < 4
            dst = lp.tile([128, N], BF16,
                          name=(f"xcs{f}" if is_xc else f"zr{f - 4}"),
                          tag=(f"xcs{f}" if is_xc else f"zr{f - 4}"))
            (xcs if is_xc else zr).append(dst)
            for n0, nl in NC2:
                ps = pm.tile([128, nl], F32, name="inmm", tag="mm")
                _matsum_t(c, ps, inT, f, xt, n0, nl, stop=not is_xc)
                if is_xc:
                    # t-1 tap: diag(cw0)@W on a token-shifted rhs slice
                    if n0 == 0:
                        for kb in range(2):
                            nc.tensor.matmul(ps[:, 1:nl],
                                             in0T[kb][f][:, :],
                                             xt[kb][:, 0:nl - 1],
                                             start=False, stop=(kb == 1))
                    else:
                        for kb in range(2):
                            nc.tensor.matmul(ps[:, :],
                                             in0T[kb][f][:, :],
                                             xt[kb][:, n0 - 1:n0 - 1 + nl],
                                             start=False, stop=(kb == 1))
                if is_xc:
                    nc.scalar.activation(dst[:, n0:n0 + nl], ps[:, :], AF.Silu,
                                         bias=cb[:, f:f + 1])
                else:
                    nc.scalar.activation(dst[:, n0:n0 + nl], ps[:, :], AF.Silu)

    if l == 0:
        _dbg(c, "xcs", [t[:, :] for t in xcs] + [t[:, :] for t in zr])
    if TRUNC:
        return xt

    # ---- x_proj -> B/C rows + dt input
    xpT = _load_mt(c, lp, f"xpT_{l}", 48, 4, tag="xpT")
    bcrows = lp.tile([32, N], BF16, name="bcrows", tag="bcrows")
    dtin = lp.tile([16, N], BF16, name="dtin", tag="dtin")
    for n0, nl in NC2:
        ps = pm.tile([48, nl], F32, name="xpmm", tag="mm")
        _matsum_t(c, ps, xpT, 0, xcs, n0, nl)
        nc.scalar.copy(bcrows[:, n0:n0 + nl], ps[:32, :])
        nc.scalar.copy(dtin[:, n0:n0 + nl], ps[32:48, :])
    bcd = c.dp.tile([1, 32 * N], BF16, name=f"bcd{l}", tag="bc_dram")
    nc.sync.dma_start(bcd[:, :], bcrows[:, :])
    if l == 0:
        _dbg(c, "bc", [bcrows[:, :], dtin[:, :]])

    # ---- dt = softplus(dtin @ dtwT + dtb) ; wT = dt * xcs
    dtwT = _load_mt(c, lp, f"dtwT_{l}", 512, 1, tag="dtwT")
    dtb = _load(c, lp, f"dtb_{l}")
    dtT, wT = [], []
    for g in range(4):
        u = rp.tile([128, N], F32, name=f"dtu{g}", tag="da", bufs=2)
        for n0, nl in NC2:
            ps = pm.tile([128, nl], F32, name="dtmm", tag="mm")
            nc.tensor.matmul(ps[:, :], dtwT[0][g][:, :],
                             dtin[:, n0:n0 + nl], start=True, stop=True)
            nc.scalar.activation(u[:, n0:n0 + nl], ps[:, :], AF.Exp,
                                 bias=dtb[:, g:g + 1])
        dt_ = lp.tile([128, N], BF16, name=f"dtT{g}", tag=f"dtT{g}")
        nc.scalar.activation(dt_[:, :], u[:, :], AF.Ln, bias=1.0)
        dtT.append(dt_)
        w_ = lp.tile([128, N], BF16, name=f"wT{g}", tag=f"wT{g}")
        nc.vector.tensor_mul(w_[:, :], dt_[:, :], xcs[g][:, :])
        wT.append(w_)

    # ---- scan: 16 states in 2 half-passes of 8; y accumulated on PE
    # (diag(D) start + identity matmuls into PSUM), PSUM evacuated to a
    # bf16 SBUF partial per half to bound PSUM/SBUF footprint.
    diagD = _load_mt(c, lp, f"diagD_{l}", 512, 1, tag="diagD")
    ySB = [lp.tile([128, N], BF16, name=f"ysb{g}", tag=f"ysb{g}")
           for g in range(4)]
    ym = []
    for h in range(2):
        Bh = rp.tile([128, 8 * N], BF16, name="Bh", tag="Bh", bufs=1)
        nc.gpsimd.dma_start(Bh[:, :],
                            bcd[0:1, h * 8 * N:(h + 1) * 8 * N]
                            .broadcast_to([128, 8 * N]))
        Ch = rp.tile([128, 8 * N], BF16, name="Ch", tag="Ch", bufs=1)
        nc.gpsimd.dma_start(Ch[:, :],
                            bcd[0:1, (16 + h * 8) * N:(24 + h * 8) * N]
                            .broadcast_to([128, 8 * N]))
        for g in range(4):
            yp = pa.tile([128, N], F32, name=f"yps{g}", tag="yps", bufs=2)
            if h == 0:
                for n0, nl in NC2:
                    nc.tensor.matmul(yp[:, n0:n0 + nl],
                                     diagD[0][g][:, :],
                                     xcs[g][:, n0:n0 + nl],
                                     start=True, stop=False)
            for si in range(8):
                s = h * 8 + si
                da = rp.tile([128, N], F32, name="da", tag="da", bufs=2)
                nc.scalar.activation(da[:, :], dtT[g][:, :], AF.Exp,
                                     scale=float(-(s + 1)))
                dbx = rp.tile([128, N], BF16, name="dbx", tag="dbx", bufs=3)
                # offload part of the elementwise muls to the idle GPSIMD
                # engine (scan itself only lowers on DVE)
                meng = nc.gpsimd if s % 2 == 0 else nc.vector
                meng.tensor_mul(dbx[:, :], wT[g][:, :],
                                Bh[:, si * N:(si + 1) * N])
                hh = rp.tile([128, N], BF16, name="h", tag="h", bufs=3)
                nc.vector.tensor_tensor_scan(hh[:, :], da[:, :], dbx[:, :],
                                             0.0, AL.mult, AL.add)
                tmp = rp.tile([128, N], BF16, name="tmp", tag="tmp", bufs=3)
                ceng = nc.gpsimd if s % 4 == 0 else nc.vector
                ceng.tensor_mul(tmp[:, :], hh[:, :],
                                Ch[:, si * N:(si + 1) * N])
                for n0, nl in NC2:
                    nc.tensor.matmul(yp[:, n0:n0 + nl], c.I128[:, :],
                                     tmp[:, n0:n0 + nl],
                                     start=(h == 1 and si == 0),
                                     stop=(si == 7))
            if h == 0:
                nc.scalar.copy(ySB[g][:, :], yp[:, :])
            else:
                y2 = rp.tile([128, N], BF16, name="y2", tag="dbx", bufs=3)
                nc.scalar.copy(y2[:, :], yp[:, :])
                ys = rp.tile([128, N], BF16, name="ys", tag="tmp", bufs=3)
                nc.vector.tensor_add(ys[:, :], ySB[g][:, :], y2[:, :])
                o = lp.tile([128, N], BF16, name=f"ym{g}", tag=f"ym{g}")
                nc.vector.tensor_mul(o[:, :], zr[g][:, :], ys[:, :])
                ym.append(o)

    if l == 0:
        _dbg(c, "dt", [t[:, :] for t in dtT])
        _dbg(c, "ym", [t[:, :] for t in ym])

    # ---- out_proj
    outT = _load_mt(c, lp, f"outT_{l}", DM, 4, tag="outT")
    fT = []
    for mi in range(2):
        t_ = lp.tile([128, N], BF16, name=f"fT{mi}", tag=f"fT{mi}")
        fT.append(t_)
        for n0, nl in NC2:
            ps = pm.tile([128, nl], F32, name="opmm", tag="mm")
            _matsum_t(c, ps, outT, mi, ym, n0, nl)
            nc.scalar.copy(t_[:, n0:n0 + nl], ps[:, :])
    if l == 0:
        _dbg(c, "f0", [t[:, :] for t in fT])

    # ---- pair exchange (AllGather bf16) overlapped with trend extraction
    fdram = c.dp.tile([256, N], BF16, name=f"fd{l}", tag="fdram")
    gdram = c.dp.tile([512, N], BF16, name=f"gd{l}", tag="gdram")
    for mi in range(2):
        nc.sync.dma_start(fdram[mi * 128:(mi + 1) * 128, :], fT[mi][:, :])
    nc.gpsimd.collective_compute("AllGather", AL.bypass, replica_groups=PAIRS,
                                 ins=[fdram.opt()], outs=[gdram.opt()])

    _trend_extract(c, l)

    G = []
    for ri in range(4):
        g_ = rp.tile([128, N], BF16, name=f"G{ri}", tag=f"G{ri % 2}", bufs=2)
        nc.sync.dma_start(g_[:, :], gdram[ri * 128:(ri + 1) * 128, :])
        G.append(g_)

    # xnew = xt + fT + rev(G0 + G1 - fT)
    xnew = []
    for mi in range(2):
        t_ = rp.tile([128, N], BF16, name="mg", tag="h", bufs=3)
        nc.vector.tensor_add(t_[:, :], G[mi][:, :], G[2 + mi][:, :])
        nc.vector.tensor_sub(t_[:, :], t_[:, :], fT[mi][:, :])
        a_ = rp.tile([128, N], BF16, name="mga", tag="tmp", bufs=3)
        nc.vector.tensor_add(a_[:, :], xt[mi][:, :], fT[mi][:, :])
        xv = lp.tile([128, N], BF16, name=f"xnew{mi}", tag=f"xnew{mi}")
        nc.vector.tensor_add(xv[:, :], a_[:, :], t_[:, ::-1])
        xnew.append(xv)

    n1w = _load(c, lp, f"n1w_{l}")
    n1b = _load(c, lp, f"n1b_{l}")
    xln = _layer_norm(c, rp, xnew, n1w, n1b, lp, f"xln{l}_")

    F1 = _load_mt(c, lp, f"f1T_{l}", DFF, 2, tag="f1T")
    F2 = _load_mt(c, lp, f"f2T_{l}", DM, 2, tag="f2T")
    f1b = _load(c, lp, f"f1b_{l}")
    f2b = _load(c, lp, f"f2b_{l}")
    h1 = []
    for mf in range(2):
        t_ = lp.tile([128, N], BF16, name=f"ffh{mf}", tag=f"ffh{mf}")
        h1.append(t_)
        for n0, nl in NC2:
            ps = pm.tile([128, nl], F32, name="f1mm", tag="mm")
            _matsum_t(c, ps, F1, mf, xln, n0, nl)
            nc.scalar.activation(t_[:, n0:n0 + nl], ps[:, :], AF.Gelu,
                                 bias=f1b[:, mf:mf + 1])
    xe2 = []
    for mi in range(2):
        y2 = rp.tile([128, N], F32, name="ffy", tag="ffy", bufs=2)
        for n0, nl in NC2:
            ps = pm.tile([128, nl], F32, name="f2mm", tag="mm")
            _matsum_t(c, ps, F2, mi, h1, n0, nl)
            nc.scalar.activation(y2[:, n0:n0 + nl], ps[:, :], AF.Identity,
                                 bias=f2b[:, mi:mi + 1])
        xv = lp.tile([128, N], BF16, name=f"xe2{mi}", tag=f"xe2{mi}")
        nc.vector.tensor_add(xv[:, :], xln[mi][:, :], y2[:, :])
        xe2.append(xv)
    n2w = _load(c, lp, f"n2w_{l}")
    n2b = _load(c, lp, f"n2b_{l}")
    xout = _layer_norm(c, rp, xe2, n2w, n2b, c.gp,
                       "xtB" if l % 2 == 0 else "xtA")
    return xout


# ---------------------------------------------------------------- entry
def _get_program():
    if "prog" not in _CACHE:
        _CACHE["prog"] = _build()
    return _CACHE["prog"]


def kernel(**inputs):
    nc = _get_program()
    in_maps = [make_core_inputs(inputs, c) for c in range(8)]
    res = run_bass_kernel_spmd(nc, in_maps, list(range(8))).results
    out = np.empty((B, H, N, 1), np.float32)
    for b in range(B):
        out[b, :, :, 0] = res[2 * b]["pred"]
    return out


if __name__ == "__main__":
    print("building program...")
    _get_program()
    print("built ok")


# revision 47
# speedup vs baseline: 1.0904x; 1.0904x over previous
"""DSTMamba Trainium2 kernel: 8 NeuronCores, SPMD. v2.

Core c handles (batch b=c//2, direction d=c%2); odd cores see the token
axis reversed so one forward-scan program serves both directions. The
bidirectional merge is a pair AllGather (bf16) + reversal-linearity
combine (symmetric SPMD, no control flow).

v2 vs v1:
- all data DMAs issued from the SP engine (HWDGE) instead of gpsimd
  (SWDGE costs ~1us of Pool time per DMA); weights host-blocked to
  [128, X] so each tensor is ONE DMA.
- depthwise causal conv folded into in_proj: host premultiplies
  diag(cw1)@W / diag(cw0)@W; the t-1 tap is a second PSUM-accumulated
  matmul on a token-shifted rhs slice. Act Silu evacuates PSUM directly.
- B/C state rows broadcast in 2 wide DMAs per half-layer ([128, 8N])
  instead of 32 per-state broadcasts.
- y = D*xc + sum_s h_s*C_s accumulated on the idle TensorEngine
  (diag(D) matmul + per-state identity matmuls into PSUM).
- pair exchange: bf16 AllGather + merge xnew = x + fT + rev(G0+G1-fT);
  tail trend extraction emitted inside the exchange gaps.
- activation stream in bf16 (weights too) for 2x DVE ops and lighter
  DMA/SBUF; dA stays f32 (bf16 would corrupt near-1 decay factors).
"""

import numpy as np

import concourse.bacc as bacc
import concourse.mybir as mybir
from concourse import tile
from concourse.bass_utils import run_bass_kernel_spmd

B, L, H, N = 4, 512, 96, 862
DM, DS = 256, 16
DI = 512
DTR = 16
DFF, NLAYERS = 256, 2
DSL, KSTD = 3, 25
EPS = 1e-5

F32 = mybir.dt.float32
F32R = mybir.dt.float32r
BF16 = mybir.dt.bfloat16
AL = mybir.AluOpType
AF = mybir.ActivationFunctionType

NC2 = [(0, 512), (512, 350)]  # psum-bank-sized moving-dim chunks of N=862
PAIRS = [[0, 1], [2, 3], [4, 5], [6, 7]]

DEBUG = False
TRUNC = False  # build only front + layer-0 in_proj (debug bisection)
_CACHE = {}


# ---------------------------------------------------------------- host math
def _mavg_matrix(length):
    M = np.zeros((length, length), np.float64)
    p = (KSTD - 1) // 2
    for i in range(length):
        for d in range(-p, p + 1):
            j = min(max(i + d, 0), length - 1)
            M[i, j] += 1.0 / KSTD
    return M


def _pool_matrix(lo, hi):
    P = np.zeros((lo, hi), np.float64)
    for i in range(lo):
        P[i, 2 * i] = 0.5
        P[i, 2 * i + 1] = 0.5
    return P


def _trend_ops():
    ops = []
    P = np.eye(L)
    cur = L
    for s in range(DSL + 1):
        ops.append(_mavg_matrix(cur) @ P)
        if s < DSL:
            P = _pool_matrix(cur // 2, cur) @ P
            cur //= 2
    return ops  # [512,512],[256,512],[128,512],[64,512]


def _col(v):
    v = np.asarray(v, np.float32).reshape(-1)
    if v.size <= 128:
        return np.ascontiguousarray(v.reshape(-1, 1))
    return np.ascontiguousarray(v.reshape(-1, 128).T)


def _row(v):
    return np.ascontiguousarray(np.asarray(v, np.float32).reshape(1, -1))


def _t(m):
    return np.ascontiguousarray(np.asarray(m, np.float32).T)


def _bf(x):
    import ml_dtypes
    return np.ascontiguousarray(np.asarray(x, dtype=ml_dtypes.bfloat16))


def _blk(wT, bf=False):
    """[K, M] lhsT -> [min(K,128), ceil(K/128)*M] column-blocked SBUF image."""
    wT = np.asarray(wT, np.float32)
    K, M = wT.shape
    if K > 128:
        assert K % 128 == 0
        wT = np.concatenate([wT[b * 128:(b + 1) * 128] for b in range(K // 128)],
                            axis=1)
    wT = np.ascontiguousarray(wT)
    return _bf(wT) if bf else wT


def make_core_inputs(inputs, core):
    b, d = core // 2, core % 2
    g = lambda k: np.asarray(inputs[k], np.float32)

    m = {}
    x = g("history_data")[b, :, :, 0]
    if d == 1:
        x = x[:, ::-1]
    m["x_in"] = _blk(np.ascontiguousarray(x))  # [128, 4N]

    tops = _trend_ops()
    m["seaop_T"] = _blk(_t(np.eye(L) - tops[0]))
    for s in range(4):
        m[f"trop{s}_T"] = _blk(_t(tops[s]))

    m["emb_lhsT"] = _blk(_t(g("emb_w")))
    m["emb_b"] = _col(g("emb_b"))

    for l in range(NLAYERS):
        W = g("m_in")[l, d]               # [1024, 256]; rows 0:512 xc, 512: z
        cw = g("m_conv_w")[l, d]          # [512, 2]
        Wxc, Wz = W[:DI], W[DI:]
        W1 = Wxc * cw[:, 1:2]             # diag(cw1) @ Wxc
        W0 = Wxc * cw[:, 0:1]             # diag(cw0) @ Wxc
        m[f"inT_{l}"] = _blk(_t(np.concatenate([W1, Wz], 0)), bf=True)
        m[f"in0T_{l}"] = _blk(_t(W0), bf=True)
        m[f"cb_{l}"] = _col(g("m_conv_b")[l, d])

        xp = g("m_xproj")[l, d]           # [48, 512]: dt 0:16, B 16:32, C 32:48
        xp_r = np.concatenate([xp[16:32], xp[32:48], xp[0:16]], 0)  # B|C|dt
        m[f"xpT_{l}"] = _blk(_t(xp_r), bf=True)                  # [128, 4*48]
        m[f"dtwT_{l}"] = _blk(_t(g("m_dt_w")[l, d]), bf=True)    # [16, 512]
        m[f"dtb_{l}"] = _col(g("m_dt_b")[l, d])

        Dv = g("m_D")[l, d]
        dg = np.zeros((128, 4 * 128), np.float32)
        for gi in range(4):
            dg[:, gi * 128:(gi + 1) * 128] = np.diag(Dv[gi * 128:(gi + 1) * 128])
        m[f"diagD_{l}"] = _bf(dg)

        m[f"outT_{l}"] = _blk(_t(g("m_out")[l, d]), bf=True)     # [128, 4*256]
        m[f"n1w_{l}"] = _col(g("n1_w")[l]); m[f"n1b_{l}"] = _col(g("n1_b")[l])
        m[f"n2w_{l}"] = _col(g("n2_w")[l]); m[f"n2b_{l}"] = _col(g("n2_b")[l])
        m[f"f1T_{l}"] = _blk(_t(g("f1_w")[l]), bf=True)          # [128, 2*256]
        m[f"f1b_{l}"] = _col(g("f1_b")[l])
        m[f"f2T_{l}"] = _blk(_t(g("f2_w")[l]), bf=True)
        m[f"f2b_{l}"] = _col(g("f2_b")[l])

    m["I128"] = _bf(np.eye(128))
    m["encnw"] = _col(g("encn_w")); m["encnb"] = _col(g("encn_b"))
    m["proj_lhsT"] = _blk(_t(g("proj_w")), bf=True)              # [128, 2*96]
    m["projb"] = _col(g("proj_b"))

    for i in range(DSL):
        m[f"u{i}w1T"] = _blk(_t(g(f"u{i}w1")), bf=True)
        m[f"u{i}b1"] = _col(g(f"u{i}b1"))
        m[f"u{i}w2T"] = _blk(_t(g(f"u{i}w2")), bf=True)
        m[f"u{i}b2"] = _col(g(f"u{i}b2"))
    for s in range(4):
        m[f"map{s}T"] = _blk(_t(g(f"map{s}_w")), bf=True)
    m["mapb"] = _col(sum(g(f"map{s}_b") for s in range(4)))

    rvw, rvb, trw = g("revin_w"), g("revin_b"), g("tre_w")
    if d == 1:
        rvw, rvb, trw = rvw[::-1], rvb[::-1], trw[::-1]
    m["rvw_row"] = _row(rvw)
    m["rvb_row"] = _row(rvb)
    m["trw_row"] = _row(trw)
    m["ones_col"] = np.ones((128, 1), np.float32)
    m["ones_bf"] = _bf(np.ones((128, 1)))
    return m


# ------------------------------------------------------------- device build
class _Ctx:
    pass


def _build():
    nc = bacc.Bacc("TRN2", target_bir_lowering=False, debug=False,
                   num_devices=8)

    def din(name, shape, dt=F32):
        return nc.dram_tensor(name, list(shape), dt, kind="ExternalInput").ap()

    I = {}
    I["x_in"] = din("x_in", [128, 4 * N], F32R)
    I["seaop_T"] = din("seaop_T", [128, 4 * L], F32R)
    for s, ls in enumerate([512, 256, 128, 64]):
        I[f"trop{s}_T"] = din(f"trop{s}_T", [128, 4 * ls], F32R)
    I["emb_lhsT"] = din("emb_lhsT", [128, 4 * DM], F32R)
    I["emb_b"] = din("emb_b", [128, DM // 128])
    for l in range(NLAYERS):
        I[f"inT_{l}"] = din(f"inT_{l}", [128, 2 * 1024], BF16)
        I[f"in0T_{l}"] = din(f"in0T_{l}", [128, 2 * 512], BF16)
        I[f"cb_{l}"] = din(f"cb_{l}", [128, DI // 128])
        I[f"xpT_{l}"] = din(f"xpT_{l}", [128, 4 * 48], BF16)
        I[f"dtwT_{l}"] = din(f"dtwT_{l}", [16, 512], BF16)
        I[f"dtb_{l}"] = din(f"dtb_{l}", [128, DI // 128])
        I[f"diagD_{l}"] = din(f"diagD_{l}", [128, 4 * 128], BF16)
        I[f"outT_{l}"] = din(f"outT_{l}", [128, 4 * DM], BF16)
        for k in ["n1w", "n1b", "n2w", "n2b", "f1b", "f2b"]:
            I[f"{k}_{l}"] = din(f"{k}_{l}", [128, DM // 128])
        I[f"f1T_{l}"] = din(f"f1T_{l}", [128, 2 * DFF], BF16)
        I[f"f2T_{l}"] = din(f"f2T_{l}", [128, 2 * DM], BF16)
    I["I128"] = din("I128", [128, 128], BF16)
    I["encnw"] = din("encnw", [128, DM // 128])
    I["encnb"] = din("encnb", [128, DM // 128])
    I["proj_lhsT"] = din("proj_lhsT", [128, 2 * H], BF16)
    I["projb"] = din("projb", [H, 1])
    for i, (li, lo) in enumerate([(64, 128), (128, 256), (256, 512)]):
        I[f"u{i}w1T"] = din(f"u{i}w1T", [min(li, 128), max(1, li // 128) * lo],
                            BF16)
        I[f"u{i}b1"] = din(f"u{i}b1", [min(lo, 128), max(1, lo // 128)])
        I[f"u{i}w2T"] = din(f"u{i}w2T", [min(lo, 128), max(1, lo // 128) * lo],
                            BF16)
        I[f"u{i}b2"] = din(f"u{i}b2", [min(lo, 128), max(1, lo // 128)])
    for s, ls in enumerate([512, 256, 128, 64]):
        I[f"map{s}T"] = din(f"map{s}T", [min(ls, 128), max(1, ls // 128) * H],
                            BF16)
    I["mapb"] = din("mapb", [H, 1])
    for k in ["rvw_row", "rvb_row", "trw_row"]:
        I[k] = din(k, [1, N])
    I["ones_col"] = din("ones_col", [128, 1], F32R)
    I["ones_bf"] = din("ones_bf", [128, 1], BF16)

    out_pred = nc.dram_tensor("pred", [H, N], F32, kind="ExternalOutput").ap()

    c = _Ctx()
    c.nc, c.I, c.out_pred = nc, I, out_pred
    c.dbg = {}
    with tile.TileContext(nc) as tc:
        c.tc = tc
        _emit(c)
    nc.compile()
    return nc


def _dbg(c, name, aps):
    if not DEBUG:
        return
    nc = c.nc
    rows = sum(a.shape[0] for a in aps)
    o = nc.dram_tensor(f"dbg_{name}", [rows, N], F32, kind="ExternalOutput").ap()
    r0 = 0
    for a in aps:
        r = a.shape[0]
        nc.gpsimd.dma_start(o[r0:r0 + r, :], a)
        r0 += r
    c.dbg[name] = o


def _load(c, pool, key, tag=None):
    ap = c.I[key]
    t_ = pool.tile(list(ap.shape), ap.dtype, name=key, tag=tag or key)
    c.nc.sync.dma_start(t_[:, :], ap[:, :])
    return t_


def _lhs(tile_, kb, mo, M, mw=128):
    """k-block kb, m-cols [mo, mo+mw) slice of a column-blocked lhsT tile.
    ONLY for f32r weights — bf16 lhsT column slices at nonzero offsets
    fault the PE; bf16 weights go through _load_mt instead."""
    return tile_[:, kb * M + mo: kb * M + mo + mw]


def _matsum_w(c, psum, wtile, M, nk, mo, mw, rhs, n0, nl, start=True, stop=True):
    """psum (+)= sum_kb lhsT(kb)[mo:mo+mw].T @ rhs[kb][:, n0:n0+nl]"""
    nc = c.nc
    for kb in range(nk):
        nc.tensor.matmul(psum[:, :], _lhs(wtile, kb, mo, M, mw),
                         rhs[kb][:, n0:n0 + nl],
                         start=(start and kb == 0), stop=(stop and kb == nk - 1))


def _load_mt(c, pool, key, M, nk, tag=None):
    """Column-blocked dram [K<=128, nk*M] -> tiles[kb][mc] of [K, <=128],
    one DMA each, so every bf16 lhsT operand sits at column offset 0."""
    ap = c.I[key]
    K = ap.shape[0]
    out = []
    for kb in range(nk):
        row = []
        for mc in range((M + 127) // 128):
            mw = min(128, M - mc * 128)
            o0 = kb * M + mc * 128
            t_ = pool.tile([K, mw], ap.dtype, name=f"{key}_{kb}_{mc}",
                           tag=f"{tag or key}_{kb}_{mc}")
            c.nc.sync.dma_start(t_[:, :], ap[:, o0:o0 + mw])
            row.append(t_)
        out.append(row)
    return out


def _matsum_t(c, psum, wt, mc, rhs, n0, nl, start=True, stop=True):
    """psum (+)= sum_kb wt[kb][mc].T @ rhs[kb][:, n0:n0+nl]"""
    nc = c.nc
    nk = len(wt)
    for kb in range(nk):
        nc.tensor.matmul(psum[:, :], wt[kb][mc][:, :],
                         rhs[kb][:, n0:n0 + nl],
                         start=(start and kb == 0), stop=(stop and kb == nk - 1))


def _bcast(c, pool, row_ap, parts, tag, via_dram=True, bufs=1, dt=F32):
    # broadcast_to (stride-0 partition) DMAs must go through SWDGE
    # (gpsimd): the HWDGE path corrupts SBUF with late/incomplete writes.
    nc = c.nc
    if via_dram:
        d = c.dp.tile([1, N], F32, name=f"bd_{tag}", tag=f"bd_{tag}")
        nc.sync.dma_start(d[:, :], row_ap.bitcast(F32))
        src = d[:, :]
    else:
        src = row_ap.bitcast(F32)
    bt = pool.tile([parts, N], dt, name=f"bc_{tag}", tag=f"bc_{tag}",
                   bufs=bufs)
    nc.gpsimd.dma_start(bt[:, :], src.broadcast_to([parts, N]))
    return bt


def _layer_norm(c, scr, xin, wcol, bcol, outpool, outtag):
    """xin: 2 [128,N] bf16 tiles -> 2 [128,N] bf16 tiles (norm over 256)."""
    nc, pm = c.nc, c.pm
    mrow = scr.tile([1, N], F32, name=f"lnm_{outtag}", tag="ln_mrow", bufs=1)
    qrow = scr.tile([1, N], F32, name=f"lnq_{outtag}", tag="ln_qrow", bufs=1)
    for n0, nl in NC2:
        ps = pm.tile([1, nl], F32, name="lnps", tag="mm1")
        for mi in range(2):
            nc.tensor.matmul(ps[:, :], c.ones_bf[:, :], xin[mi][:, n0:n0 + nl],
                             start=(mi == 0), stop=(mi == 1))
        nc.scalar.activation(mrow[:, n0:n0 + nl], ps[:, :], AF.Copy,
                             scale=1.0 / DM)
        ps2 = pm.tile([1, nl], F32, name="lnps2", tag="mm1")
        for mi in range(2):
            sq = scr.tile([128, N], BF16, name="lnsq", tag="sq", bufs=2)
            nc.scalar.activation(sq[:, n0:n0 + nl],
                                 xin[mi][:, n0:n0 + nl], AF.Square)
            nc.tensor.matmul(ps2[:, :], c.ones_bf[:, :], sq[:, n0:n0 + nl],
                             start=(mi == 0), stop=(mi == 1))
        nc.scalar.activation(qrow[:, n0:n0 + nl], ps2[:, :], AF.Copy,
                             scale=1.0 / DM)
    tmp_ = scr.tile([1, N], F32, name=f"lnt_{outtag}", tag="ln_trow", bufs=1)
    nc.vector.tensor_mul(tmp_[:, :], mrow[:, :], mrow[:, :])
    nc.vector.tensor_sub(qrow[:, :], qrow[:, :], tmp_[:, :])
    nc.scalar.activation(qrow[:, :], qrow[:, :], AF.Ln, bias=c.epscol[:1, :])
    nc.scalar.activation(qrow[:, :], qrow[:, :], AF.Exp, scale=-0.5)
    mb = _bcast(c, scr, mrow[:, :], 128, "lnm", dt=BF16)
    rb = _bcast(c, scr, qrow[:, :], 128, "lnr", dt=BF16)
    out = []
    for mi in range(2):
        o = outpool.tile([128, N], BF16, name=f"{outtag}{mi}", tag=f"{outtag}{mi}")
        d1 = scr.tile([128, N], BF16, name="lnd1", tag="d1", bufs=2)
        nc.vector.tensor_sub(d1[:, :], xin[mi][:, :], mb[:, :])
        nc.vector.tensor_mul(d1[:, :], d1[:, :], rb[:, :])
        nc.vector.tensor_scalar(o[:, :], d1[:, :],
                                wcol[:, mi:mi + 1],
                                bcol[:, mi:mi + 1], AL.mult, AL.add)
        out.append(o)
    return out


def _emit(c):
    nc, tc, I = c.nc, c.tc, c.I
    import contextlib
    with contextlib.ExitStack() as est:
        gp = est.enter_context(tc.tile_pool(name="glob", bufs=1))
        pm = est.enter_context(tc.tile_pool(name="pmm", bufs=2, space="PSUM"))
        dp = est.enter_context(tc.tile_pool(name="drm", bufs=1, space="DRAM"))
        c.gp, c.pm, c.dp = gp, pm, dp

        c.ones_col = _load(c, gp, "ones_col")
        c.ones_bf = _load(c, gp, "ones_bf")
        c.I128 = _load(c, gp, "I128")
        epscol = gp.tile([128, 1], F32, name="epscol", tag="epscol")
        c.nc.gpsimd.memset(epscol[:, :], EPS)
        c.epscol = epscol
        r_mean = gp.tile([1, N], F32, name="r_mean", tag="r_mean")
        r_sc = gp.tile([1, N], F32, name="r_sc", tag="r_sc")
        c.r_mean, c.r_sc = r_mean, r_sc

        # ======================================================== front
        with tc.tile_pool(name="front", bufs=1) as fp:
            r_std = fp.tile([1, N], F32, name="r_std", tag="r_std")
            r_wr = fp.tile([1, N], F32, name="r_wr", tag="r_wr")
            r_msq = fp.tile([1, N], F32, name="r_msq", tag="r_msq")
            Xw = _load(c, fp, "x_in")

            def Xs(ci, a, b):
                return Xw[:, ci * N + a: ci * N + b]

            for n0, nl in NC2:
                ps = pm.tile([1, nl], F32, name="rvs", tag="mm1")
                for ci in range(4):
                    nc.tensor.matmul(ps[:, :], c.ones_col[:, :],
                                     Xs(ci, n0, n0 + nl),
                                     start=(ci == 0), stop=(ci == 3))
                nc.scalar.activation(r_mean[:, n0:n0 + nl], ps[:, :],
                                     AF.Copy, scale=1.0 / L)
                ps2 = pm.tile([1, nl], F32, name="rvq", tag="mm1")
                for ci in range(4):
                    sq = fp.tile([128, N], F32R, name="rvsq", tag="fsq", bufs=2)
                    nc.scalar.activation(sq[:, n0:n0 + nl],
                                         Xs(ci, n0, n0 + nl).bitcast(F32),
                                         AF.Square)
                    nc.tensor.matmul(ps2[:, :], c.ones_col[:, :],
                                     sq[:, n0:n0 + nl],
                                     start=(ci == 0), stop=(ci == 3))
                nc.scalar.activation(r_msq[:, n0:n0 + nl], ps2[:, :],
                                     AF.Copy, scale=1.0 / L)
            nc.vector.tensor_mul(r_wr[:, :], r_mean[:, :], r_mean[:, :])
            nc.vector.tensor_sub(r_msq[:, :], r_msq[:, :], r_wr[:, :])
            nc.scalar.activation(r_msq[:, :], r_msq[:, :], AF.Ln,
                                 bias=c.epscol[:1, :])
            nc.scalar.activation(r_std[:, :], r_msq[:, :], AF.Exp, scale=0.5)
            nc.scalar.activation(r_wr[:, :], r_msq[:, :], AF.Exp, scale=-0.5)
            rvw = fp.tile([1, N], F32, name="rvwrow", tag="rvwrow")
            nc.sync.dma_start(rvw[:, :], I["rvw_row"][:, :])
            nc.vector.tensor_mul(r_wr[:, :], r_wr[:, :], rvw[:, :])
            t1 = fp.tile([1, N], F32, name="sct1", tag="sct1")
            nc.vector.tensor_scalar_add(t1[:, :], rvw[:, :], 1e-10)
            nc.vector.reciprocal(t1[:, :], t1[:, :])
            nc.vector.tensor_mul(r_sc[:, :], t1[:, :], r_std[:, :])

            mb = _bcast(c, fp, r_mean[:, :], 128, "rvm")
            wb = _bcast(c, fp, r_wr[:, :], 128, "rvw")
            bb = _bcast(c, fp, I["rvb_row"], 128, "rvb", via_dram=False)
            c.xn = []
            for ci in range(4):
                o = gp.tile([128, N], F32R, name=f"xn{ci}", tag=f"xn{ci}")
                d1 = fp.tile([128, N], F32, name="rvd", tag="rvd", bufs=2)
                nc.vector.tensor_sub(d1[:, :], Xs(ci, 0, N).bitcast(F32),
                                     mb[:, :])
                nc.vector.tensor_mul(d1[:, :], d1[:, :], wb[:, :])
                nc.vector.tensor_add(o[:, :], d1[:, :], bb[:, :])
                c.xn.append(o)
            _dbg(c, "xn", [t[:, :].bitcast(F32) for t in c.xn])

            SE = _load(c, fp, "seaop_T")
            xsea = []
            for mc in range(4):
                t_ = fp.tile([128, N], F32R, name=f"xsea{mc}", tag=f"xsea{mc}")
                xsea.append(t_)
                for n0, nl in NC2:
                    ps = pm.tile([128, nl], F32, name="semm", tag="mm")
                    _matsum_w(c, ps, SE, L, 4, mc * 128, 128, c.xn, n0, nl)
                    nc.scalar.copy(t_[:, n0:n0 + nl], ps[:, :])
            EL = _load(c, fp, "emb_lhsT")
            emb_b = _load(c, fp, "emb_b")
            xt = []
            for mc in range(2):
                t_ = gp.tile([128, N], BF16, name=f"xtA{mc}", tag=f"xtA{mc}")
                xt.append(t_)
                for n0, nl in NC2:
                    ps = pm.tile([128, nl], F32, name="embmm", tag="mm")
                    _matsum_w(c, ps, EL, DM, 4, mc * 128, 128, xsea, n0, nl)
                    nc.scalar.activation(t_[:, n0:n0 + nl], ps[:, :],
                                         AF.Identity,
                                         bias=emb_b[:, mc:mc + 1])
            _dbg(c, "x0", [t[:, :] for t in xt])

        # ======================================================== encoder
        c.trt = None
        for l in range(1 if TRUNC else NLAYERS):
            with contextlib.ExitStack() as lst:
                lp = lst.enter_context(tc.tile_pool(name=f"lay{l}", bufs=1))
                rp = lst.enter_context(tc.tile_pool(name=f"rot{l}", bufs=2))
                pa = lst.enter_context(
                    tc.tile_pool(name=f"pda{l}", bufs=2, space="PSUM"))
                xt = _mamba_layer(c, l, lp, rp, pa, xt)
                if l == 0:
                    _dbg(c, "xl0", [t[:, :] for t in xt])

        # ======================================================== tail
        if TRUNC:
            return
        with contextlib.ExitStack() as tst:
            tp = tst.enter_context(tc.tile_pool(name="tail", bufs=1))
            encw = _load(c, tp, "encnw")
            encb = _load(c, tp, "encnb")
            xf = _layer_norm(c, tp, xt, encw, encb, c.gp, "xtB")
            PRJ = _load_mt(c, tp, "proj_lhsT", H, 2)
            projb = _load(c, tp, "projb")
            seaT = tp.tile([H, N], F32, name="seaT", tag="seaT")
            for n0, nl in NC2:
                ps = pm.tile([H, nl], F32, name="prmm", tag="mm")
                _matsum_t(c, ps, PRJ, 0, xf, n0, nl)
                nc.scalar.activation(seaT[:, n0:n0 + nl], ps[:, :], AF.Identity,
                                     bias=projb[:, :])
            _dbg(c, "sea", [seaT[:, :]])

            tr0, tr1, tr2, tr3 = c.trt
            o1, o2 = c.mix_o1, c.mix_o2
            o3 = _mixstep(c, tp, o2, 2, tr0)

            outst = [o3, o2, o1, tr3]
            MP = [_load_mt(c, tp, f"map{s}T", H, len(outst[s]))
                  for s in range(4)]
            mapb = _load(c, tp, "mapb")
            treT = tp.tile([H, N], F32, name="treT", tag="treT")
            for n0, nl in NC2:
                ps = pm.tile([H, nl], F32, name="mpmm", tag="mm")
                ops = []
                for s in range(4):
                    for kb in range(len(outst[s])):
                        ops.append((MP[s][kb][0], outst[s][kb]))
                for i, (w_, x_) in enumerate(ops):
                    nc.tensor.matmul(ps[:, :], w_[:, :], x_[:, n0:n0 + nl],
                                     start=(i == 0), stop=(i == len(ops) - 1))
                nc.scalar.activation(treT[:, n0:n0 + nl], ps[:, :], AF.Identity,
                                     bias=mapb[:, :])
            _dbg(c, "tre", [treT[:, :]])

            p1 = tp.tile([H, N], F32, name="fin1", tag="fin1")
            twb = _bcast(c, tp, I["trw_row"], H, "finb", via_dram=False)
            nc.vector.tensor_mul(p1[:, :], treT[:, :], twb[:, :])
            nc.vector.tensor_add(p1[:, :], p1[:, :], seaT[:, :])
            rbb = _bcast(c, tp, I["rvb_row"], H, "finb", via_dram=False)
            nc.vector.tensor_sub(p1[:, :], p1[:, :], rbb[:, :])
            scb = _bcast(c, tp, c.r_sc[:, :], H, "finb")
            nc.vector.tensor_mul(p1[:, :], p1[:, :], scb[:, :])
            mnb = _bcast(c, tp, c.r_mean[:, :], H, "finb")
            nc.vector.tensor_add(p1[:, :], p1[:, :], mnb[:, :])
            nc.sync.dma_start(c.out_pred[:, :], p1[:, :])


def _mixstep(c, gtpool, low, i, high):
    """TimeMixer trend mixing step i: high += W2 @ gelu(W1 @ low + b1) + b2."""
    nc, pm = c.nc, c.pm
    with c.tc.tile_pool(name=f"wu{i}", bufs=1) as wu:
        nk1 = len(low)
        lo_cols = c.I[f"u{i}w1T"].shape[1] // nk1
        W1 = _load_mt(c, wu, f"u{i}w1T", lo_cols, nk1)
        b1 = _load(c, wu, f"u{i}b1")
        W2 = _load_mt(c, wu, f"u{i}w2T", lo_cols, (lo_cols + 127) // 128)
        b2 = _load(c, wu, f"u{i}b2")
        gt = []
        for mc in range((lo_cols + 127) // 128):
            parts = min(128, lo_cols - mc * 128)
            g_ = gtpool.tile([parts, N], BF16, name=f"mxg{i}_{mc}",
                             tag=f"gA{mc}")
            gt.append(g_)
            for n0, nl in NC2:
                ps = pm.tile([parts, nl], F32, name="mxmm", tag="mm")
                _matsum_t(c, ps, W1, mc, low, n0, nl)
                nc.scalar.activation(g_[:, n0:n0 + nl], ps[:, :], AF.Gelu,
                                     bias=b1[:parts, mc:mc + 1])
        out = []
        for mc in range(len(high)):
            parts = high[mc].shape[0]
            o_ = high[mc]
            out.append(o_)
            for n0, nl in NC2:
                ps = pm.tile([parts, nl], F32, name="mxmm2", tag="mm")
                _matsum_t(c, ps, W2, mc, gt, n0, nl)
                b_ = gtpool.tile([parts, N], F32, name="mxb", tag="mxb",
                                 bufs=1)
                nc.scalar.activation(b_[:, n0:n0 + nl], ps[:, :], AF.Identity,
                                     bias=b2[:parts, mc:mc + 1])
                nc.vector.tensor_add(o_[:, n0:n0 + nl], o_[:, n0:n0 + nl],
                                     b_[:, n0:n0 + nl])
        return out


def _trend_extract(c, l):
    """Emit tail work that depends only on c.xn inside the exchange gaps.
    Layer 0's gap: trend scales 1..3. Layer 1's: scale 0 + mixsteps 0,1."""
    nc = c.nc
    if c.trt is None:
        c.trt = [None] * 4
    for s, ls in ([(1, 256), (2, 128), (3, 64)] if l == 0 else [(0, 512)]):
        with c.tc.tile_pool(name=f"wtr{s}", bufs=1) as wtr:
            TR = _load(c, wtr, f"trop{s}_T")
            mt = []
            for mc in range((ls + 127) // 128):
                parts = min(128, ls - mc * 128)
                t_ = c.gp.tile([parts, N], BF16, name=f"tr{s}_{mc}",
                               tag=f"tr{s}_{mc}")
                mt.append(t_)
                for n0, nl in NC2:
                    ps = c.pm.tile([parts, nl], F32, name="trmm", tag="mm")
                    _matsum_w(c, ps, TR, ls, 4, mc * 128, parts, c.xn, n0, nl)
                    nc.scalar.copy(t_[:, n0:n0 + nl], ps[:, :])
            c.trt[s] = mt
    if l == 1:
        c.mix_o1 = _mixstep(c, c.gp, c.trt[3], 0, c.trt[2])
        c.mix_o2 = _mixstep(c, c.gp, c.mix_o1, 1, c.trt[1])


def _mamba_layer(c, l, lp, rp, pa, xt):
    nc, pm = c.nc, c.pm

    # ---- in_proj with folded conv; Act Silu evacuates PSUM directly
    xcs, zr = [], []
    with c.tc.tile_pool(name=f"w1_{l}", bufs=1) as wp1:
        inT = _load_mt(c, wp1, f"inT_{l}", 1024, 2, tag="inT")
        in0T = _load_mt(c, wp1, f"in0T_{l}", 512, 2, tag="in0T")
        cb = _load(c, lp, f"cb_{l}")
        for f in range(8):
            is_xc = f < 4
            dst = lp.tile([128, N], BF16,
                          name=(f"xcs{f}" if is_xc else f"zr{f - 4}"),
                          tag=(f"xcs{f}" if is_xc else f"zr{f - 4}"))
            (xcs if is_xc else zr).append(dst)
            for n0, nl in NC2:
                ps = pm.tile([128, nl], F32, name="inmm", tag="mm")
                _matsum_t(c, ps, inT, f, xt, n0, nl, stop=not is_xc)
                if is_xc:
                    # t-1 tap: diag(cw0)@W on a token-shifted rhs slice
                    if n0 == 0:
                        for kb in range(2):
                            nc.tensor.matmul(ps[:, 1:nl],
                                             in0T[kb][f][:, :],
                                             xt[kb][:, 0:nl - 1],
                                             start=False, stop=(kb == 1))
                    else:
                        for kb in range(2):
                            nc.tensor.matmul(ps[:, :],
                                             in0T[kb][f][:, :],
                                             xt[kb][:, n0 - 1:n0 - 1 + nl],
                                             start=False, stop=(kb == 1))
                if is_xc:
                    nc.scalar.activation(dst[:, n0:n0 + nl], ps[:, :], AF.Silu,
                                         bias=cb[:, f:f + 1])
                else:
                    nc.scalar.activation(dst[:, n0:n0 + nl], ps[:, :], AF.Silu)

    if l == 0:
        _dbg(c, "xcs", [t[:, :] for t in xcs] + [t[:, :] for t in zr])
    if TRUNC:
        return xt

    # ---- x_proj -> B/C rows + dt input
    xpT = _load_mt(c, lp, f"xpT_{l}", 48, 4, tag="xpT")
    bcrows = lp.tile([32, N], BF16, name="bcrows", tag="bcrows")
    dtin = lp.tile([16, N], BF16, name="dtin", tag="dtin")
    for n0, nl in NC2:
        ps = pm.tile([48, nl], F32, name="xpmm", tag="mm")
        _matsum_t(c, ps, xpT, 0, xcs, n0, nl)
        nc.scalar.copy(bcrows[:, n0:n0 + nl], ps[:32, :])
        nc.scalar.copy(dtin[:, n0:n0 + nl], ps[32:48, :])
    bcd = c.dp.tile([1, 32 * N], BF16, name=f"bcd{l}", tag="bc_dram")
    nc.sync.dma_start(bcd[:, :], bcrows[:, :])
    if l == 0:
        _dbg(c, "bc", [bcrows[:, :], dtin[:, :]])

    # ---- dt = softplus(dtin @ dtwT + dtb) ; wT = dt * xcs
    dtwT = _load_mt(c, lp, f"dtwT_{l}", 512, 1, tag="dtwT")
    dtb = _load(c, lp, f"dtb_{l}")
    dtT, wT = [], []
    for g in range(4):
        u = rp.tile([128, N], F32, name=f"dtu{g}", tag="da", bufs=3)
        for n0, nl in NC2:
            ps = pm.tile([128, nl], F32, name="dtmm", tag="mm")
            nc.tensor.matmul(ps[:, :], dtwT[0][g][:, :],
                             dtin[:, n0:n0 + nl], start=True, stop=True)
            nc.scalar.activation(u[:, n0:n0 + nl], ps[:, :], AF.Exp,
                                 bias=dtb[:, g:g + 1])
        dt_ = lp.tile([128, N], BF16, name=f"dtT{g}", tag=f"dtT{g}")
        nc.scalar.activation(dt_[:, :], u[:, :], AF.Ln, bias=1.0)
        dtT.append(dt_)
        w_ = lp.tile([128, N], BF16, name=f"wT{g}", tag=f"wT{g}")
        nc.vector.tensor_mul(w_[:, :], dt_[:, :], xcs[g][:, :])
        wT.append(w_)

    # ---- scan: 16 states in 2 half-passes of 8; y accumulated on PE
    # (diag(D) start + identity matmuls into PSUM), PSUM evacuated to a
    # bf16 SBUF partial per half to bound PSUM/SBUF footprint.
    diagD = _load_mt(c, lp, f"diagD_{l}", 512, 1, tag="diagD")
    ySB = [lp.tile([128, N], BF16, name=f"ysb{g}", tag=f"ysb{g}")
           for g in range(4)]
    ym = []
    for h in range(2):
        Bh = rp.tile([128, 8 * N], BF16, name="Bh", tag="Bh", bufs=1)
        nc.gpsimd.dma_start(Bh[:, :],
                            bcd[0:1, h * 8 * N:(h + 1) * 8 * N]
                            .broadcast_to([128, 8 * N]))
        Ch = rp.tile([128, 8 * N], BF16, name="Ch", tag="Ch", bufs=1)
        nc.gpsimd.dma_start(Ch[:, :],
                            bcd[0:1, (16 + h * 8) * N:(24 + h * 8) * N]
                            .broadcast_to([128, 8 * N]))
        for g in range(4):
            yp = pa.tile([128, N], F32, name=f"yps{g}", tag="yps", bufs=2)
            if h == 0:
                for n0, nl in NC2:
                    nc.tensor.matmul(yp[:, n0:n0 + nl],
                                     diagD[0][g][:, :],
                                     xcs[g][:, n0:n0 + nl],
                                     start=True, stop=False)
            for si in range(8):
                s = h * 8 + si
                da = rp.tile([128, N], F32, name="da", tag="da", bufs=3)
                nc.scalar.activation(da[:, :], dtT[g][:, :], AF.Exp,
                                     scale=float(-(s + 1)))
                dbx = rp.tile([128, N], BF16, name="dbx", tag="dbx", bufs=3)
                # offload part of the elementwise muls to the idle GPSIMD
                # engine (scan itself only lowers on DVE)
                meng = nc.gpsimd if s % 2 == 0 else nc.vector
                meng.tensor_mul(dbx[:, :], wT[g][:, :],
                                Bh[:, si * N:(si + 1) * N])
                hh = rp.tile([128, N], BF16, name="h", tag="h", bufs=3)
                nc.vector.tensor_tensor_scan(hh[:, :], da[:, :], dbx[:, :],
                                             0.0, AL.mult, AL.add)
                tmp = rp.tile([128, N], BF16, name="tmp", tag="tmp", bufs=3)
                ceng = nc.gpsimd if s % 4 == 0 else nc.vector
                ceng.tensor_mul(tmp[:, :], hh[:, :],
                                Ch[:, si * N:(si + 1) * N])
                for n0, nl in NC2:
                    nc.tensor.matmul(yp[:, n0:n0 + nl], c.I128[:, :],
                                     tmp[:, n0:n0 + nl],
                                     start=(h == 1 and si == 0),
                                     stop=(si == 7))
            if h == 0:
                nc.scalar.copy(ySB[g][:, :], yp[:, :])
            else:
                y2 = rp.tile([128, N], BF16, name="y2", tag="dbx", bufs=3)
                nc.scalar.copy(y2[:, :], yp[:, :])
                ys = rp.tile([128, N], BF16, name="ys", tag="tmp", bufs=3)
                nc.vector.tensor_add(ys[:, :], ySB[g][:, :], y2[:, :])
                o = lp.tile([128, N], BF16, name=f"ym{g}", tag=f"ym{g}")
                nc.vector.tensor_mul(o[:, :], zr[g][:, :], ys[:, :])
                ym.append(o)

    if l == 0:
        _dbg(c, "dt", [t[:, :] for t in dtT])
        _dbg(c, "ym", [t[:, :] for t in ym])

    # ---- out_proj
    outT = _load_mt(c, lp, f"outT_{l}", DM, 4, tag="outT")
    fT = []
    for mi in range(2):
        t_ = lp.tile([128, N], BF16, name=f"fT{mi}", tag=f"fT{mi}")
        fT.append(t_)
        for n0, nl in NC2:
            ps = pm.tile([128, nl], F32, name="opmm", tag="mm")
            _matsum_t(c, ps, outT, mi, ym, n0, nl)
            nc.scalar.copy(t_[:, n0:n0 + nl], ps[:, :])
    if l == 0:
        _dbg(c, "f0", [t[:, :] for t in fT])

    # ---- pair exchange (AllGather bf16) overlapped with trend extraction
    fdram = c.dp.tile([256, N], BF16, name=f"fd{l}", tag="fdram")
    gdram = c.dp.tile([512, N], BF16, name=f"gd{l}", tag="gdram")
    for mi in range(2):
        nc.sync.dma_start(fdram[mi * 128:(mi + 1) * 128, :], fT[mi][:, :])
    nc.gpsimd.collective_compute("AllGather", AL.bypass, replica_groups=PAIRS,
                                 ins=[fdram.opt()], outs=[gdram.opt()])

    _trend_extract(c, l)

    G = []
    for ri in range(4):
        g_ = rp.tile([128, N], BF16, name=f"G{ri}", tag=f"G{ri % 2}", bufs=2)
        nc.sync.dma_start(g_[:, :], gdram[ri * 128:(ri + 1) * 128, :])
        G.append(g_)

    # xnew = xt + fT + rev(G0 + G1 - fT)
    xnew = []
    for mi in range(2):
        t_ = rp.tile([128, N], BF16, name="mg", tag="h", bufs=3)
        nc.vector.tensor_add(t_[:, :], G[mi][:, :], G[2 + mi][:, :])
        nc.vector.tensor_sub(t_[:, :], t_[:, :], fT[mi][:, :])
        a_ = rp.tile([128, N], BF16, name="mga", tag="tmp", bufs=3)
        nc.vector.tensor_add(a_[:, :], xt[mi][:, :], fT[mi][:, :])
        xv = lp.tile([128, N], BF16, name=f"xnew{mi}", tag=f"ysb{mi}")
        nc.vector.tensor_add(xv[:, :], a_[:, :], t_[:, ::-1])
        xnew.append(xv)

    n1w = _load(c, lp, f"n1w_{l}")
    n1b = _load(c, lp, f"n1b_{l}")
    xln = _layer_norm(c, rp, xnew, n1w, n1b, lp, f"xln{l}_")

    F1 = _load_mt(c, lp, f"f1T_{l}", DFF, 2, tag="f1T")
    F2 = _load_mt(c, lp, f"f2T_{l}", DM, 2, tag="f2T")
    f1b = _load(c, lp, f"f1b_{l}")
    f2b = _load(c, lp, f"f2b_{l}")
    h1 = []
    for mf in range(2):
        t_ = lp.tile([128, N], BF16, name=f"ffh{mf}", tag=f"ffh{mf}")
        h1.append(t_)
        for n0, nl in NC2:
            ps = pm.tile([128, nl], F32, name="f1mm", tag="mm")
            _matsum_t(c, ps, F1, mf, xln, n0, nl)
            nc.scalar.activation(t_[:, n0:n0 + nl], ps[:, :], AF.Gelu,
                                 bias=f1b[:, mf:mf + 1])
    xe2 = []
    for mi in range(2):
        y2 = rp.tile([128, N], F32, name="ffy", tag="ffy", bufs=2)
        for n0, nl in NC2:
            ps = pm.tile([128, nl], F32, name="f2mm", tag="mm")
            _matsum_t(c, ps, F2, mi, h1, n0, nl)
            nc.scalar.activation(y2[:, n0:n0 + nl], ps[:, :], AF.Identity,
                                 bias=f2b[:, mi:mi + 1])
        xv = lp.tile([128, N], BF16, name=f"xe2{mi}", tag=f"xe2{mi}")
        nc.vector.tensor_add(xv[:, :], xln[mi][:, :], y2[:, :])
        xe2.append(xv)
    n2w = _load(c, lp, f"n2w_{l}")
    n2b = _load(c, lp, f"n2b_{l}")
    xout = _layer_norm(c, rp, xe2, n2w, n2b, c.gp,
                       "xtB" if l % 2 == 0 else "xtA")
    return xout


# ---------------------------------------------------------------- entry
def _get_program():
    if "prog" not in _CACHE:
        _CACHE["prog"] = _build()
    return _CACHE["prog"]


def kernel(**inputs):
    nc = _get_program()
    in_maps = [make_core_inputs(inputs, c) for c in range(8)]
    res = run_bass_kernel_spmd(nc, in_maps, list(range(8))).results
    out = np.empty((B, H, N, 1), np.float32)
    for b in range(B):
        out[b, :, :, 0] = res[2 * b]["pred"]
    return out


if __name__ == "__main__":
    print("building program...")
    _get_program()
    print("built ok")
